# revision 25
# baseline (speedup 1.0000x reference)
"""Trainium2 Bass kernel for nn_Att_Bilinear_layer2_keycat_textual_visual.

Math (full shapes B=32,N=64,A=32,O=32,D=512,QD=512):
    v      = einsum('bnao,bod->bnad', att1, obj_reps) + t_rep
    inter  = einsum('bnq,qd->bnd', q[:,:,0,:], W)
    logits = einsum('bnd,bnad->bna', inter, v) + bias
    s      = softmax((logits/t)*m) * m ; att2 = s / (sum_a s + 1e-13*z)
    out    = einsum('bna,bnao->bno', att2, att1)

Restructured to avoid materializing v:
    logits[b,n,a] = t_rep[b,n,a,:].inter[b,n,:] + att1[b,n,a,:].s1[b,n,:]
    where s1[b,n,o] = inter[b,n,:].obj_reps[b,o,:]

Sharding: data-parallel over batch b (4 of 32 per core, 8 cores), W
replicated. No collectives.

Two HBM-traffic reductions on the dominant t_rep stream (the kernel is
DMA-bound):
  * fp16 everywhere on the logits path (10 mantissa bits; measured end-to-end
    rel-err ~2e-3 vs the fp32 oracle, far inside the 2e-2 gate).
  * mask-aware compaction: att2 is exactly zero wherever tags_attention==0
    (the reference's renormalized masked softmax), so t_rep[b,n,a,:] with
    tags[b,n,a]==0 never influences the output. The host keeps only unmasked
    a-slots per token ("j-slots"), sorts tokens within each batch by their
    unmasked count k, and pads each 16-token rank-window to a global width
    Kbar[window]. Padded slots carry zeros, which reproduces the reference's
    exp(0-max) contribution for masked entries in the z-denominator.

On-device per core (BL=4 batches, TOK=256 tokens, sorted order):
  interT[d,tok]  = W^T q^T/t        (PE fp16, fp32 PSUM accum, 4 d-blocks)
  s1T[o,tok]     = objT^T interT    (PE fp16)
  Big pass: 2 group-tiles G of 128 tokens (32-token group G of each batch).
  For each half h, one [128,16*Kbar] PSUM bank packs the 4 batches via
  column-tiled matmuls (tile_position=(0,32b)):
      psq[32b+r, (n',j)] = sum_d interT[d, (b,G) tokens] t_repc[d, n', j]
                         + sum_o s1T[o, (b,G) tokens] att1Tc[o, n', j]
  The block-diagonal (r == n'+16h) is extracted with a constant-mask
  multiply + strided reduce on full 128-partition tiles (DVE), halves summed
  -> logits [128, 32 j-slots]. Masked softmax per row (DVE+ACT exp with
  z-accum). Final einsum att2 x att1 (j-space) as broadcast-mult + strided
  reduce (DVE). Output [256,32] fp32 per core, un-permuted on host.
"""

import sys

if "/opt/trn_rl_repo" not in sys.path:
    sys.path.insert(0, "/opt/trn_rl_repo")

from contextlib import ExitStack

import numpy as np

import concourse.bacc as bacc
import concourse.mybir as mybir
import concourse.tile as tile
from concourse.bass_utils import run_bass_kernel_spmd

B, N, A, O, D, QD = 32, 64, 32, 32, 512, 512
NCORES = 8
BL = B // NCORES          # batches per core
TOK = BL * N              # tokens per core
F32 = mybir.dt.float32
F16 = mybir.dt.float16


def _build(bias_over_t: float, kprof: tuple, reps: int = 1):
    # kprof = (Kbar[G0h0], Kbar[G0h1], Kbar[G1h0], Kbar[G1h1]), even ints
    K = list(kprof)
    offs = np.concatenate([[0], np.cumsum([16 * k for k in K])]).tolist()
    CW = offs[-1]          # compacted t_rep cols per (batch, d-chunk)

    nc = bacc.Bacc("TRN2", target_bir_lowering=False, debug=False,
                   num_devices=NCORES)

    # cst0 = [w_sw | q_sw | obj_sw], cst1 = [att1n | dm | m16] — one DMA each
    C0 = 4 * D + 4 * TOK + 4 * BL * O
    C1 = 2 * A * O + CW + 2 * A
    t_repc = nc.dram_tensor("t_repc", [BL, 128, 4 * CW], F16,
                            kind="ExternalInput").ap()
    cst0 = nc.dram_tensor("cst0", [128, C0], F16, kind="ExternalInput").ap()
    cst1 = nc.dram_tensor("cst1", [128, C1], F16, kind="ExternalInput").ap()
    att1T = nc.dram_tensor("att1T", [O, BL * CW], F16,
                           kind="ExternalInput").ap()
    out = nc.dram_tensor("out", [TOK, O], F32, kind="ExternalOutput").ap()

    with tile.TileContext(nc) as tc, ExitStack() as ctx:
      cpool = ctx.enter_context(tc.tile_pool(name="const", bufs=2))
      tpool = ctx.enter_context(tc.tile_pool(name="trep", bufs=8))
      ipool = ctx.enter_context(tc.tile_pool(name="psumI", bufs=2,
                                             space="PSUM"))
      qpool = ctx.enter_context(tc.tile_pool(name="psumQ", bufs=1,
                                             space="PSUM"))
      spool = ctx.enter_context(tc.tile_pool(name="work", bufs=3))
      for rep in range(reps):

        # ---- input loads, issue-ordered for overlap: cst0 (needed by inter)
        # first, then t_rep b0, att1T (needed by the first psq chains), the
        # remaining t_rep batches, and cst1 (only needed at extraction).
        c0_sb = cpool.tile([128, C0], F16, tag="c0")
        nc.sync.dma_start(c0_sb[:], cst0)
        w_sb = [c0_sb[:, D * c:D * (c + 1)] for c in range(4)]
        q_sb = [c0_sb[:, 4 * D + TOK * c:4 * D + TOK * (c + 1)]
                for c in range(4)]
        o0 = 4 * D + 4 * TOK
        obj_sb = [c0_sb[:, o0 + BL * O * c:o0 + BL * O * (c + 1)]
                  for c in range(4)]

        t_sb = []
        for b in range(BL):
            t_ = tpool.tile([128, 4 * CW], F16, tag="trep")
            t_sb.append(t_)
        nc.sync.dma_start(t_sb[0][:], t_repc[0])

        a1T_all = cpool.tile([O, BL * CW], F16, tag="a1T_all")
        nc.sync.dma_start(a1T_all[:], att1T)

        for b in range(1, BL):
            nc.sync.dma_start(t_sb[b][:], t_repc[b])

        c1_sb = cpool.tile([128, C1], F16, tag="c1")
        nc.sync.dma_start(c1_sb[:], cst1)
        a1n_sb = [c1_sb[:, A * O * j:A * O * (j + 1)] for j in range(2)]
        dm_sb = [c1_sb[:, 2 * A * O + offs[w]:2 * A * O + offs[w + 1]]
                 for w in range(4)]
        m0 = 2 * A * O + CW
        m_sb = [c1_sb[:, m0 + A * j:m0 + A * (j + 1)] for j in range(2)]

        # ---- interT[d, tok] = (q/t @ W)^T, in 4 d-blocks of 128 ----
        interT_sb = []
        for m in range(4):
            ps = ipool.tile([128, TOK], F32, tag="ps_inter")
            for c in range(4):
                nc.tensor.matmul(
                    ps[:], w_sb[c][:, 128 * m:128 * (m + 1)], q_sb[c][:],
                    start=(c == 0), stop=(c == 3),
                )
            it = cpool.tile([128, TOK], F16, tag=f"interT{m}")
            nc.scalar.copy(it[:], ps[:])
            interT_sb.append(it)

        # ---- s1T[o, tok] = obj_reps . inter / t ----
        ps1 = ipool.tile([O, TOK], F32, tag="ps_s1")
        for b in range(BL):
            for c in range(4):
                nc.tensor.matmul(
                    ps1[:, 64 * b:64 * (b + 1)],
                    obj_sb[c][:, O * b:O * (b + 1)],
                    interT_sb[c][:, 64 * b:64 * (b + 1)],
                    start=(c == 0), stop=(c == 3),
                )
        s1T_sb = cpool.tile([O, TOK], F16, tag="s1T")
        nc.scalar.copy(s1T_sb[:], ps1[:])

        # ---- big pass: logits via packed column-tiled block-diag matmuls.
        # Batch-outer order: each t_rep tile is consumed in one PE burst
        # right after its DMA lands, so the next rep's loads unblock early.
        ot = spool.tile([128, 2 * O], F32, tag="ot")
        # full-bank tiles: accumulation-group clears are per PSUM bank
        psq = [qpool.tile([128, 512], F32, tag=f"psq{w}",
                          name=f"psq_{rep}_{w}")[:, :16 * K[w]]
               for w in range(4)]
        for b in range(4):
            for w in range(4):
                G, h = divmod(w, 2)
                W16 = 16 * K[w]
                tk = 64 * b + 32 * G          # first token of group (b, G)
                for c in range(4):
                    sl = slice(CW * c + offs[w], CW * c + offs[w] + W16)
                    nc.tensor.matmul(
                        psq[w][32 * b:32 * (b + 1), :],
                        interT_sb[c][:, tk:tk + 32],
                        t_sb[b][:, sl],
                        start=(c == 0), stop=False,
                        tile_position=(0, 32 * b),
                    )
                sl = slice(CW * b + offs[w], CW * b + offs[w] + W16)
                nc.tensor.matmul(
                    psq[w][32 * b:32 * (b + 1), :],
                    s1T_sb[:, tk:tk + 32],
                    a1T_all[:, sl],
                    start=False, stop=True,
                    tile_position=(0, 32 * b),
                )

        rd = []
        for w in range(4):
            Kb = K[w]
            W16 = 16 * Kb
            msk = spool.tile([128, W16], F16, tag=f"msk{w % 2}")
            nc.vector.tensor_mul(msk[:], psq[w][:], dm_sb[w][:])
            r = spool.tile([128, A], F32, tag=f"red{w}")
            if Kb < A:
                nc.scalar.memzero(r[:, Kb:])
            nc.vector.reduce_sum(
                r[:, :Kb], msk[:].rearrange("p (n j) -> p j n", j=Kb),
                axis=mybir.AxisListType.X,
            )
            rd.append(r)

        for G in range(2):
            lg = spool.tile([128, A], F32, tag="lg")
            nc.vector.tensor_add(lg[:], rd[2 * G][:], rd[2 * G + 1][:])

            # ---- softmax + final einsum for this 128-token tile ----
            lm = spool.tile([128, A], F32, tag="lm")
            if bias_over_t != 0.0:
                nc.vector.scalar_tensor_tensor(
                    lm[:], lg[:], bias_over_t, m_sb[G][:],
                    op0=mybir.AluOpType.add, op1=mybir.AluOpType.mult)
            else:
                nc.vector.tensor_mul(lm[:], lg[:], m_sb[G][:])
            negmax = spool.tile([128, 1], F32, tag="negmax")
            nc.vector.reduce_max(negmax[:], lm[:], axis=mybir.AxisListType.X,
                                 negate=True)
            e = spool.tile([128, A], F32, tag="e")
            z = spool.tile([128, 1], F32, tag="z")
            nc.scalar.activation(e[:], lm[:], mybir.ActivationFunctionType.Exp,
                                 bias=negmax[:], scale=1.0, accum_out=z[:])
            em = spool.tile([128, A], F32, tag="em")
            nc.vector.tensor_mul(em[:], e[:], m_sb[G][:])
            ssum = spool.tile([128, 1], F32, tag="ssum")
            nc.vector.reduce_sum(ssum[:], em[:], axis=mybir.AxisListType.X)
            den = spool.tile([128, 1], F32, tag="den")
            nc.vector.tensor_scalar(
                den[:], z[:], 1e-13, ssum[:],
                op0=mybir.AluOpType.mult, op1=mybir.AluOpType.add,
            )
            rcp = spool.tile([128, 1], F32, tag="rcp")
            nc.vector.reciprocal(rcp[:], den[:])
            att2 = spool.tile([128, A], F16, tag="att2")
            nc.vector.tensor_scalar_mul(att2[:], em[:], rcp[:])

            prod = spool.tile([128, A * O], F16, tag="prod")
            nc.vector.tensor_mul(
                prod[:].rearrange("p (a o) -> p a o", a=A),
                a1n_sb[G][:].rearrange("p (a o) -> p a o", a=A),
                att2[:].unsqueeze(2).broadcast_to([128, A, O]),
            )
            nc.vector.reduce_sum(
                ot[:, O * G:O * (G + 1)],
                prod[:].rearrange("p (a o) -> p o a", a=A),
                axis=mybir.AxisListType.X,
            )
        # store on the ACT HWDGE ring so it can't head-of-line block the
        # next rep's input loads on the sync-engine ring
        nc.scalar.dma_start(out.rearrange("(g p) o -> p g o", g=2), ot[:])

    nc.compile()
    return nc


def _kprof(tags):
    """Global per-rank-window padded widths from the full tags tensor."""
    k = np.asarray(tags).reshape(B, N, A).sum(-1)     # [B, N]
    ks = np.sort(k, axis=1)                           # ascending per batch
    K = []
    for w in range(4):
        kb = int(ks[:, 16 * w:16 * (w + 1)].max())
        K.append(max(2, (kb + 1) // 2 * 2))           # even, >=2
    return tuple(K)


def _shard_inputs(q, att1, obj_reps, tags_attention, t_rep, W, t, kprof):
    K = list(kprof)
    offs = np.concatenate([[0], np.cumsum([16 * kk for kk in K])]).tolist()
    CW = offs[-1]

    C0 = 4 * D + 4 * TOK + 4 * BL * O
    C1 = 2 * A * O + CW + 2 * A

    w_sw = np.ascontiguousarray(
        W.reshape(4, 128, D).transpose(1, 0, 2).reshape(128, 4 * D)
    ).astype(np.float16)

    # dm[p, off(w) + n'*Kbar + j] = 1 iff n' == p%32 - 16*h(w)
    dm = np.zeros((128, CW), np.float16)
    for w in range(4):
        h = w % 2
        for p in range(128):
            nrel = p % 32 - 16 * h
            if 0 <= nrel < 16:
                o0 = offs[w] + nrel * K[w]
                dm[p, o0:o0 + K[w]] = 1.0

    kk = np.asarray(tags_attention).reshape(B, N, A).sum(-1)    # [B, N]
    perm = np.argsort(kk, axis=1, kind="stable")                # [B, N]

    in_maps = []
    meta = []
    for i in range(NCORES):
        bs = range(BL * i, BL * (i + 1))
        qf_l, trc_l, a1T_l, a1n_l, m_l = [], [], [], [], []
        aidx_core = []
        for b in bs:
            pm = perm[b]
            tags_s = np.asarray(tags_attention[b])[pm]          # [N, A] sorted
            k_s = tags_s.sum(-1)                                # [N]
            # j-slot -> a index per token (pad slot -> A, a zero column)
            aidx = np.full((N, A + 1), A, np.int64)
            for p in range(N):
                nz = np.nonzero(tags_s[p])[0]
                aidx[p, :len(nz)] = nz
            aidx_core.append((pm, k_s, aidx))

            # column tables for the compacted blocks
            colmap_pos = np.empty(CW, np.int64)
            colmap_a = np.empty(CW, np.int64)
            for w in range(4):
                G, h = divmod(w, 2)
                base = 32 * G + 16 * h
                cols = np.arange(16 * K[w])
                npr, j = divmod(cols, K[w])
                colmap_pos[offs[w]:offs[w + 1]] = base + npr
                colmap_a[offs[w]:offs[w + 1]] = aidx[base + npr, j]

            trp = np.concatenate(
                [np.asarray(t_rep[b])[pm],
                 np.zeros((N, 1, D), np.float32)], axis=1)      # [N, A+1, D]
            a1p = np.concatenate(
                [np.asarray(att1[b])[pm],
                 np.zeros((N, 1, O), np.float32)], axis=1)      # [N, A+1, O]

            tcols = trp[colmap_pos, colmap_a, :]                # [CW, D]
            trc_l.append(tcols.T.reshape(4, 128, CW)
                         .transpose(1, 0, 2).reshape(128, 4 * CW))
            a1T_l.append(a1p[colmap_pos, colmap_a, :].T)        # [O, CW]

            qf_l.append((np.asarray(q[b, :, 0, :])[pm] / float(t)))  # [N, QD]

            # att1n in j-space [N, A, O]; mask m' [N, A]
            a1n_l.append(a1p[np.arange(N)[:, None], aidx[:, :A], :])
            m_l.append((np.arange(A)[None, :] < k_s[:, None]))

        qf = np.concatenate(qf_l, 0)                            # [TOK, QD]
        q_sw = np.ascontiguousarray(
            qf.T.reshape(4, 128, TOK).transpose(1, 0, 2).reshape(128, 4 * TOK)
        ).astype(np.float16)
        obj_sw = np.ascontiguousarray(
            np.asarray(obj_reps[BL * i:BL * (i + 1)])
            .transpose(2, 0, 1).reshape(4, 128, BL * O)
            .transpose(1, 0, 2).reshape(128, 4 * BL * O)
        ).astype(np.float16)
        trc = np.ascontiguousarray(np.stack(trc_l, 0)).astype(np.float16)
        a1T = np.ascontiguousarray(np.concatenate(a1T_l, 1)).astype(np.float16)

        # row order for the two 128-token tiles: row 128G+p <-> token
        # (b=p//32, pos=32G+p%32)
        a1n_tok = np.stack(a1n_l, 0)                            # [BL, N, A, O]
        m_tok = np.stack(m_l, 0).astype(np.float16)             # [BL, N, A]
        cst1 = np.empty((128, C1), np.float16)
        for G in range(2):
            p = np.arange(128)
            bb, rr = p // 32, 32 * G + p % 32
            cst1[:, A * O * G:A * O * (G + 1)] = (
                a1n_tok[bb, rr].reshape(128, A * O))
            cst1[:, 2 * A * O + CW + A * G:2 * A * O + CW + A * (G + 1)] = (
                m_tok[bb, rr])
        cst1[:, 2 * A * O:2 * A * O + CW] = dm

        cst0 = np.empty((128, C0), np.float16)
        cst0[:, :4 * D] = w_sw
        cst0[:, 4 * D:4 * D + 4 * TOK] = q_sw
        cst0[:, 4 * D + 4 * TOK:] = obj_sw

        in_maps.append({
            "t_repc": trc,
            "cst0": np.ascontiguousarray(cst0),
            "cst1": np.ascontiguousarray(cst1),
            "att1T": a1T,
        })
        meta.append([x[0] for x in aidx_core])                  # perms
    return in_maps, meta


def _unshard_out(res, meta):
    full = np.empty((B, N, O), np.float32)
    for i in range(NCORES):
        o = np.asarray(res.results[i]["out"])                   # [TOK, O]
        perms = meta[i]
        for G in range(2):
            rows = o[128 * G:128 * (G + 1)]
            p = np.arange(128)
            bb, rr = p // 32, 32 * G + p % 32
            for b in range(BL):
                sel = bb == b
                full[BL * i + b, perms[b][rr[sel]], :] = rows[sel]
    return full


_NC_CACHE = {}


def _get_nc(bias_over_t: float, kprof: tuple, reps: int = 1):
    key = (float(bias_over_t), tuple(kprof), int(reps))
    if key not in _NC_CACHE:
        _NC_CACHE[key] = _build(key[0], key[1], reps=key[2])
    return _NC_CACHE[key]


def _run(inputs, trace=False, **kw):
    q = np.asarray(inputs["q"], np.float32)
    att1 = np.asarray(inputs["att1"], np.float32)
    obj_reps = np.asarray(inputs["obj_reps"], np.float32)
    tags = np.asarray(inputs["tags_attention"])
    t_rep = np.asarray(inputs["t_rep"], np.float32)
    W = np.asarray(inputs["W"], np.float32)
    bias = float(np.asarray(inputs["bias"]))
    t = float(np.asarray(inputs["t"]))

    kprof = _kprof(tags)
    nc = _get_nc(bias / t, kprof)
    in_maps, meta = _shard_inputs(q, att1, obj_reps, tags, t_rep, W, t, kprof)
    res = run_bass_kernel_spmd(nc, in_maps, core_ids=list(range(NCORES)),
                               trace=trace, **kw)
    return _unshard_out(res, meta), res


def kernel(**inputs):
    full, _ = _run(inputs, trace=False)
    return full


# revision 31
# speedup vs baseline: 1.0162x; 1.0162x over previous
"""Trainium2 Bass kernel for nn_Att_Bilinear_layer2_keycat_textual_visual.

Math (full shapes B=32,N=64,A=32,O=32,D=512,QD=512):
    v      = einsum('bnao,bod->bnad', att1, obj_reps) + t_rep
    inter  = einsum('bnq,qd->bnd', q[:,:,0,:], W)
    logits = einsum('bnd,bnad->bna', inter, v) + bias
    s      = softmax((logits/t)*m) * m ; att2 = s / (sum_a s + 1e-13*z)
    out    = einsum('bna,bnao->bno', att2, att1)

Restructured to avoid materializing v:
    logits[b,n,a] = t_rep[b,n,a,:].inter[b,n,:] + att1[b,n,a,:].s1[b,n,:]
    where s1[b,n,o] = inter[b,n,:].obj_reps[b,o,:]

Sharding: data-parallel over batch b (4 of 32 per core, 8 cores), W
replicated. No collectives.

Two HBM-traffic reductions on the dominant t_rep stream (the kernel is
DMA-bound):
  * fp16 everywhere on the logits path (10 mantissa bits; measured end-to-end
    rel-err ~2e-3 vs the fp32 oracle, far inside the 2e-2 gate).
  * mask-aware compaction: att2 is exactly zero wherever tags_attention==0
    (the reference's renormalized masked softmax), so t_rep[b,n,a,:] with
    tags[b,n,a]==0 never influences the output. The host keeps only unmasked
    a-slots per token ("j-slots"), sorts tokens within each batch by their
    unmasked count k, and pads each 16-token rank-window to a global width
    Kbar[window]. Padded slots carry zeros, which reproduces the reference's
    exp(0-max) contribution for masked entries in the z-denominator.

On-device per core (BL=4 batches, TOK=256 tokens, sorted order):
  interT[d,tok]  = W^T q^T/t        (PE fp16, fp32 PSUM accum, 4 d-blocks)
  s1T[o,tok]     = objT^T interT    (PE fp16)
  Big pass: 2 group-tiles G of 128 tokens (32-token group G of each batch).
  For each half h, one [128,16*Kbar] PSUM bank packs the 4 batches via
  column-tiled matmuls (tile_position=(0,32b)):
      psq[32b+r, (n',j)] = sum_d interT[d, (b,G) tokens] t_repc[d, n', j]
                         + sum_o s1T[o, (b,G) tokens] att1Tc[o, n', j]
  The block-diagonal (r == n'+16h) is extracted with a constant-mask
  multiply + strided reduce on full 128-partition tiles (DVE), halves summed
  -> logits [128, 32 j-slots]. Masked softmax per row (DVE+ACT exp with
  z-accum). Final einsum att2 x att1 (j-space) as broadcast-mult + strided
  reduce (DVE). Output [256,32] fp32 per core, un-permuted on host.
"""

import sys

if "/opt/trn_rl_repo" not in sys.path:
    sys.path.insert(0, "/opt/trn_rl_repo")

from contextlib import ExitStack

import numpy as np

import concourse.bacc as bacc
import concourse.mybir as mybir
import concourse.tile as tile
from concourse.bass_utils import run_bass_kernel_spmd

B, N, A, O, D, QD = 32, 64, 32, 32, 512, 512
NCORES = 8
BL = B // NCORES          # batches per core
TOK = BL * N              # tokens per core
F32 = mybir.dt.float32
F16 = mybir.dt.float16


def _build(bias_over_t: float, kprof: tuple, reps: int = 1):
    # kprof = (Kbar[G0h0], Kbar[G0h1], Kbar[G1h0], Kbar[G1h1]), even ints
    K = list(kprof)
    offs = np.concatenate([[0], np.cumsum([16 * k for k in K])]).tolist()
    CW = offs[-1]          # compacted t_rep cols per (batch, d-chunk)

    nc = bacc.Bacc("TRN2", target_bir_lowering=False, debug=False,
                   num_devices=NCORES)

    # cst0 = [w_sw | q_sw | obj_sw], cst1 = [att1n | dm(fp8) | m16] — one DMA
    # each. att1n keeps only KX = max(K) j-slots; the 0/1 diag mask is stored
    # as fp8e4 bytes reinterpreted as fp16 columns (CW/2 of them).
    KX = max(K)
    C0 = 4 * D + 4 * TOK + 4 * BL * O
    C1 = 2 * KX * O + CW // 2 + 2 * A
    t_repc = nc.dram_tensor("t_repc", [BL, 128, 4 * CW], F16,
                            kind="ExternalInput").ap()
    cst0 = nc.dram_tensor("cst0", [128, C0], F16, kind="ExternalInput").ap()
    cst1 = nc.dram_tensor("cst1", [128, C1], F16, kind="ExternalInput").ap()
    att1T = nc.dram_tensor("att1T", [O, BL * CW], F16,
                           kind="ExternalInput").ap()
    out = nc.dram_tensor("out", [TOK, O], F32, kind="ExternalOutput").ap()

    with tile.TileContext(nc) as tc, ExitStack() as ctx:
      cpool = ctx.enter_context(tc.tile_pool(name="const", bufs=2))
      tpool = ctx.enter_context(tc.tile_pool(name="trep", bufs=8))
      ipool = ctx.enter_context(tc.tile_pool(name="psumI", bufs=2,
                                             space="PSUM"))
      qpool = ctx.enter_context(tc.tile_pool(name="psumQ", bufs=1,
                                             space="PSUM"))
      spool = ctx.enter_context(tc.tile_pool(name="work", bufs=3))
      for rep in range(reps):

        # ---- input loads, issue-ordered for overlap: cst0 (needed by inter)
        # first, then t_rep b0, att1T (needed by the first psq chains), the
        # remaining t_rep batches, and cst1 (only needed at extraction).
        c0_sb = cpool.tile([128, C0], F16, tag="c0")
        nc.sync.dma_start(c0_sb[:], cst0)
        w_sb = [c0_sb[:, D * c:D * (c + 1)] for c in range(4)]
        q_sb = [c0_sb[:, 4 * D + TOK * c:4 * D + TOK * (c + 1)]
                for c in range(4)]
        o0 = 4 * D + 4 * TOK
        obj_sb = [c0_sb[:, o0 + BL * O * c:o0 + BL * O * (c + 1)]
                  for c in range(4)]

        t_sb = []
        for b in range(BL):
            t_ = tpool.tile([128, 4 * CW], F16, tag="trep")
            t_sb.append(t_)
        nc.sync.dma_start(t_sb[0][:], t_repc[0])

        a1T_all = cpool.tile([O, BL * CW], F16, tag="a1T_all")
        nc.sync.dma_start(a1T_all[:], att1T)

        for b in range(1, BL):
            nc.sync.dma_start(t_sb[b][:], t_repc[b])

        c1_sb = cpool.tile([128, C1], F16, tag="c1")
        nc.sync.dma_start(c1_sb[:], cst1)
        a1n_sb = [c1_sb[:, KX * O * j:KX * O * (j + 1)] for j in range(2)]
        d0 = 2 * KX * O
        dm_sb = [c1_sb[:, d0 + offs[w] // 2:d0 + offs[w + 1] // 2]
                 .bitcast(mybir.dt.float8e4) for w in range(4)]
        m0 = 2 * KX * O + CW // 2
        m_sb = [c1_sb[:, m0 + A * j:m0 + A * (j + 1)] for j in range(2)]

        # ---- interT[d, tok] = (q/t @ W)^T, in 4 d-blocks of 128 ----
        interT_sb = []
        for m in range(4):
            ps = ipool.tile([128, TOK], F32, tag="ps_inter")
            for c in range(4):
                nc.tensor.matmul(
                    ps[:], w_sb[c][:, 128 * m:128 * (m + 1)], q_sb[c][:],
                    start=(c == 0), stop=(c == 3),
                )
            it = cpool.tile([128, TOK], F16, tag=f"interT{m}")
            nc.scalar.copy(it[:], ps[:])
            interT_sb.append(it)

        # ---- s1T[o, tok] = obj_reps . inter / t ----
        ps1 = ipool.tile([O, TOK], F32, tag="ps_s1")
        for b in range(BL):
            for c in range(4):
                nc.tensor.matmul(
                    ps1[:, 64 * b:64 * (b + 1)],
                    obj_sb[c][:, O * b:O * (b + 1)],
                    interT_sb[c][:, 64 * b:64 * (b + 1)],
                    start=(c == 0), stop=(c == 3),
                )
        s1T_sb = cpool.tile([O, TOK], F16, tag="s1T")
        nc.scalar.copy(s1T_sb[:], ps1[:])

        # ---- big pass: logits via packed column-tiled block-diag matmuls.
        # Batch-outer order: each t_rep tile is consumed in one PE burst
        # right after its DMA lands, so the next rep's loads unblock early.
        ot = spool.tile([128, 2 * O], F32, tag="ot")
        # full-bank tiles: accumulation-group clears are per PSUM bank
        psq = [qpool.tile([128, 512], F32, tag=f"psq{w}",
                          name=f"psq_{rep}_{w}")[:, :16 * K[w]]
               for w in range(4)]
        for b in range(4):
            for w in range(4):
                G, h = divmod(w, 2)
                W16 = 16 * K[w]
                tk = 64 * b + 32 * G          # first token of group (b, G)
                for c in range(4):
                    sl = slice(CW * c + offs[w], CW * c + offs[w] + W16)
                    nc.tensor.matmul(
                        psq[w][32 * b:32 * (b + 1), :],
                        interT_sb[c][:, tk:tk + 32],
                        t_sb[b][:, sl],
                        start=(c == 0), stop=False,
                        tile_position=(0, 32 * b),
                    )
                sl = slice(CW * b + offs[w], CW * b + offs[w] + W16)
                nc.tensor.matmul(
                    psq[w][32 * b:32 * (b + 1), :],
                    s1T_sb[:, tk:tk + 32],
                    a1T_all[:, sl],
                    start=False, stop=True,
                    tile_position=(0, 32 * b),
                )

        rd = []
        for w in range(4):
            Kb = K[w]
            W16 = 16 * Kb
            msk = spool.tile([128, W16], F16, tag=f"msk{w % 2}")
            nc.vector.tensor_mul(msk[:], psq[w][:], dm_sb[w][:])
            r = spool.tile([128, A], F32, tag=f"red{w}")
            if Kb < A:
                nc.scalar.memzero(r[:, Kb:])
            nc.vector.reduce_sum(
                r[:, :Kb], msk[:].rearrange("p (n j) -> p j n", j=Kb),
                axis=mybir.AxisListType.X,
            )
            rd.append(r)

        for G in range(2):
            lg = spool.tile([128, A], F32, tag="lg")
            nc.vector.tensor_add(lg[:], rd[2 * G][:], rd[2 * G + 1][:])

            # ---- softmax + final einsum for this 128-token tile ----
            lm = spool.tile([128, A], F32, tag="lm")
            if bias_over_t != 0.0:
                nc.vector.scalar_tensor_tensor(
                    lm[:], lg[:], bias_over_t, m_sb[G][:],
                    op0=mybir.AluOpType.add, op1=mybir.AluOpType.mult)
            else:
                nc.vector.tensor_mul(lm[:], lg[:], m_sb[G][:])
            negmax = spool.tile([128, 1], F32, tag="negmax")
            nc.vector.reduce_max(negmax[:], lm[:], axis=mybir.AxisListType.X,
                                 negate=True)
            e = spool.tile([128, A], F32, tag="e")
            z = spool.tile([128, 1], F32, tag="z")
            nc.scalar.activation(e[:], lm[:], mybir.ActivationFunctionType.Exp,
                                 bias=negmax[:], scale=1.0, accum_out=z[:])
            em = spool.tile([128, A], F32, tag="em")
            nc.vector.tensor_mul(em[:], e[:], m_sb[G][:])
            ssum = spool.tile([128, 1], F32, tag="ssum")
            nc.vector.reduce_sum(ssum[:], em[:], axis=mybir.AxisListType.X)
            den = spool.tile([128, 1], F32, tag="den")
            nc.vector.tensor_scalar(
                den[:], z[:], 1e-13, ssum[:],
                op0=mybir.AluOpType.mult, op1=mybir.AluOpType.add,
            )
            rcp = spool.tile([128, 1], F32, tag="rcp")
            nc.vector.reciprocal(rcp[:], den[:])
            att2 = spool.tile([128, A], F16, tag="att2")
            nc.vector.tensor_scalar_mul(att2[:], em[:], rcp[:])

            prod = spool.tile([128, KX * O], F16, tag="prod")
            nc.vector.tensor_mul(
                prod[:].rearrange("p (a o) -> p a o", a=KX),
                a1n_sb[G][:].rearrange("p (a o) -> p a o", a=KX),
                att2[:, :KX].unsqueeze(2).broadcast_to([128, KX, O]),
            )
            nc.vector.reduce_sum(
                ot[:, O * G:O * (G + 1)],
                prod[:].rearrange("p (a o) -> p o a", a=KX),
                axis=mybir.AxisListType.X,
            )
        # store on the ACT HWDGE ring so it can't head-of-line block the
        # next rep's input loads on the sync-engine ring
        nc.scalar.dma_start(out.rearrange("(g p) o -> p g o", g=2), ot[:])

    nc.compile()
    return nc


def _kprof(tags):
    """Global per-rank-window padded widths from the full tags tensor."""
    k = np.asarray(tags).reshape(B, N, A).sum(-1)     # [B, N]
    ks = np.sort(k, axis=1)                           # ascending per batch
    K = []
    for w in range(4):
        kb = int(ks[:, 16 * w:16 * (w + 1)].max())
        K.append(max(2, (kb + 1) // 2 * 2))           # even, >=2
    return tuple(K)


def _shard_inputs(q, att1, obj_reps, tags_attention, t_rep, W, t, kprof):
    K = list(kprof)
    offs = np.concatenate([[0], np.cumsum([16 * kk for kk in K])]).tolist()
    CW = offs[-1]

    KX = max(K)
    C0 = 4 * D + 4 * TOK + 4 * BL * O
    C1 = 2 * KX * O + CW // 2 + 2 * A

    w_sw = np.ascontiguousarray(
        W.reshape(4, 128, D).transpose(1, 0, 2).reshape(128, 4 * D)
    ).astype(np.float16)

    # dm[p, off(w) + n'*Kbar + j] = 1 iff n' == p%32 - 16*h(w), stored as
    # fp8e4 bytes (1.0 == 0x38) viewed as fp16 columns
    dm = np.zeros((128, CW), np.uint8)
    for w in range(4):
        h = w % 2
        for p in range(128):
            nrel = p % 32 - 16 * h
            if 0 <= nrel < 16:
                o0 = offs[w] + nrel * K[w]
                dm[p, o0:o0 + K[w]] = 0x38
    dm16 = dm.view(np.float16)                       # [128, CW//2]

    kk = np.asarray(tags_attention).reshape(B, N, A).sum(-1)    # [B, N]
    perm = np.argsort(kk, axis=1, kind="stable")                # [B, N]

    in_maps = []
    meta = []
    for i in range(NCORES):
        bs = range(BL * i, BL * (i + 1))
        qf_l, trc_l, a1T_l, a1n_l, m_l = [], [], [], [], []
        aidx_core = []
        for b in bs:
            pm = perm[b]
            tags_s = np.asarray(tags_attention[b])[pm]          # [N, A] sorted
            k_s = tags_s.sum(-1)                                # [N]
            # j-slot -> a index per token (pad slot -> A, a zero column)
            aidx = np.full((N, A + 1), A, np.int64)
            for p in range(N):
                nz = np.nonzero(tags_s[p])[0]
                aidx[p, :len(nz)] = nz
            aidx_core.append((pm, k_s, aidx))

            # column tables for the compacted blocks
            colmap_pos = np.empty(CW, np.int64)
            colmap_a = np.empty(CW, np.int64)
            for w in range(4):
                G, h = divmod(w, 2)
                base = 32 * G + 16 * h
                cols = np.arange(16 * K[w])
                npr, j = divmod(cols, K[w])
                colmap_pos[offs[w]:offs[w + 1]] = base + npr
                colmap_a[offs[w]:offs[w + 1]] = aidx[base + npr, j]

            trp = np.concatenate(
                [np.asarray(t_rep[b])[pm],
                 np.zeros((N, 1, D), np.float32)], axis=1)      # [N, A+1, D]
            a1p = np.concatenate(
                [np.asarray(att1[b])[pm],
                 np.zeros((N, 1, O), np.float32)], axis=1)      # [N, A+1, O]

            tcols = trp[colmap_pos, colmap_a, :]                # [CW, D]
            trc_l.append(tcols.T.reshape(4, 128, CW)
                         .transpose(1, 0, 2).reshape(128, 4 * CW))
            a1T_l.append(a1p[colmap_pos, colmap_a, :].T)        # [O, CW]

            qf_l.append((np.asarray(q[b, :, 0, :])[pm] / float(t)))  # [N, QD]

            # att1n in j-space [N, KX, O]; mask m' [N, A]
            a1n_l.append(a1p[np.arange(N)[:, None], aidx[:, :KX], :])
            m_l.append((np.arange(A)[None, :] < k_s[:, None]))

        qf = np.concatenate(qf_l, 0)                            # [TOK, QD]
        q_sw = np.ascontiguousarray(
            qf.T.reshape(4, 128, TOK).transpose(1, 0, 2).reshape(128, 4 * TOK)
        ).astype(np.float16)
        obj_sw = np.ascontiguousarray(
            np.asarray(obj_reps[BL * i:BL * (i + 1)])
            .transpose(2, 0, 1).reshape(4, 128, BL * O)
            .transpose(1, 0, 2).reshape(128, 4 * BL * O)
        ).astype(np.float16)
        trc = np.ascontiguousarray(np.stack(trc_l, 0)).astype(np.float16)
        a1T = np.ascontiguousarray(np.concatenate(a1T_l, 1)).astype(np.float16)

        # row order for the two 128-token tiles: row 128G+p <-> token
        # (b=p//32, pos=32G+p%32)
        a1n_tok = np.stack(a1n_l, 0)                            # [BL, N, KX, O]
        m_tok = np.stack(m_l, 0).astype(np.float16)             # [BL, N, A]
        cst1 = np.empty((128, C1), np.float16)
        m0 = 2 * KX * O + CW // 2
        for G in range(2):
            p = np.arange(128)
            bb, rr = p // 32, 32 * G + p % 32
            cst1[:, KX * O * G:KX * O * (G + 1)] = (
                a1n_tok[bb, rr].reshape(128, KX * O))
            cst1[:, m0 + A * G:m0 + A * (G + 1)] = m_tok[bb, rr]
        cst1[:, 2 * KX * O:2 * KX * O + CW // 2] = dm16

        cst0 = np.empty((128, C0), np.float16)
        cst0[:, :4 * D] = w_sw
        cst0[:, 4 * D:4 * D + 4 * TOK] = q_sw
        cst0[:, 4 * D + 4 * TOK:] = obj_sw

        in_maps.append({
            "t_repc": trc,
            "cst0": np.ascontiguousarray(cst0),
            "cst1": np.ascontiguousarray(cst1),
            "att1T": a1T,
        })
        meta.append([x[0] for x in aidx_core])                  # perms
    return in_maps, meta


def _unshard_out(res, meta):
    full = np.empty((B, N, O), np.float32)
    for i in range(NCORES):
        o = np.asarray(res.results[i]["out"])                   # [TOK, O]
        perms = meta[i]
        for G in range(2):
            rows = o[128 * G:128 * (G + 1)]
            p = np.arange(128)
            bb, rr = p // 32, 32 * G + p % 32
            for b in range(BL):
                sel = bb == b
                full[BL * i + b, perms[b][rr[sel]], :] = rows[sel]
    return full


_NC_CACHE = {}


def _get_nc(bias_over_t: float, kprof: tuple, reps: int = 1):
    key = (float(bias_over_t), tuple(kprof), int(reps))
    if key not in _NC_CACHE:
        _NC_CACHE[key] = _build(key[0], key[1], reps=key[2])
    return _NC_CACHE[key]


def _run(inputs, trace=False, **kw):
    q = np.asarray(inputs["q"], np.float32)
    att1 = np.asarray(inputs["att1"], np.float32)
    obj_reps = np.asarray(inputs["obj_reps"], np.float32)
    tags = np.asarray(inputs["tags_attention"])
    t_rep = np.asarray(inputs["t_rep"], np.float32)
    W = np.asarray(inputs["W"], np.float32)
    bias = float(np.asarray(inputs["bias"]))
    t = float(np.asarray(inputs["t"]))

    kprof = _kprof(tags)
    nc = _get_nc(bias / t, kprof)
    in_maps, meta = _shard_inputs(q, att1, obj_reps, tags, t_rep, W, t, kprof)
    res = run_bass_kernel_spmd(nc, in_maps, core_ids=list(range(NCORES)),
                               trace=trace, **kw)
    return _unshard_out(res, meta), res


def kernel(**inputs):
    full, _ = _run(inputs, trace=False)
    return full
